# revision 44
# baseline (speedup 1.0000x reference)
"""Bass/Tile multi-head attention kernel for TRN2 (v2, all-bf16 datapath).

Per-core problem (core c handles batch b=c//2, head-group g=c%2):
  inputs:  xq, xk, xv [S, DIN] f32      (batch b slices of q/k/v)
           wq, wk, wv [DIN, DC] f32     (column slice for this head group)
           wo [DC, DOUT] f32            (row slice)
           bq, bk, bv [DC] f32
  output:  out [S, DOUT] f32  partial:  host sums the two head-group partials
           per batch and adds bo.

Key structure (H=8 local heads, depth=64, P=128):
  - x / weights enter SBUF as bf16 via gpsimd casting DMAs; x is transposed
    by the DMA XBAR (dma_start_transpose), so the PE does no transposes.
  - KT/QT are packed 2-heads-per-128-partition blocks; per-head matmuls use
    64-partition views at base 0/64 (contraction over depth=64).
  - ST:  st[k, q] = KT_h^T @ QT_h        (N=512 moving)
  - exp: split between ACT (exact, cols 0:QA) and DVE (Schraudolph int16
    bit-trick -> bf16 bits, cols QA:512), software-pipelined pdepth deep.
  - OT:  ot[d+1, q] += V_aug_h^T @ ex    (V stationary; row 64 of V_aug is
         ones -> softmax denominator in ot[64, :])
  - normalize: reciprocal + partition_broadcast + multiply, written into
    OTn [dc, q] bf16 (plain tensor_copy shifts partition base, HW-proven).
  - V-projection chunks are deferred into attention head 0 of sqt 0 as PE
    fillers; qproj(sqt+1)/outproj(sqt-1) fill between later heads.
"""

from contextlib import ExitStack

import concourse.mybir as mybir
from concourse import bacc
from concourse.tile import TileContext

F32 = mybir.dt.float32
BF16 = mybir.dt.bfloat16
I16 = mybir.dt.int16
P = 128
EXP = mybir.ActivationFunctionType.Exp
IDENT = mybir.ActivationFunctionType.Identity
COPYF = mybir.ActivationFunctionType.Copy
LOG2E = 1.4426950408889634


def build_mha_core(S=2048, DIN=1024, DC=512, DOUT=1024, H=8, depth=64,
                   SQT=512, KG=1, QA=352, num_devices=1,
                   st_bufs=4, ex_bufs=6, ot_bufs=2, gen_bufs=2, pdepth=5,
                   dbg=False):
    assert DC == H * depth and DC % P == 0 and DIN % P == 0 and S % SQT == 0
    NKT = S // P          # key chunks of 128
    NDIN = DIN // P       # input-dim k-tiles
    NDO = DC // P         # d_core blocks (2 heads each)
    NSQT = S // SQT       # attention q tiles
    NQC = SQT // P        # 128-wide q chunks per sqt
    NKG = NKT // KG
    scale = 1.0 / float(depth) ** 0.5
    a_exp = 128.0 * LOG2E * scale
    b_exp = 16250.4

    nc = bacc.Bacc("TRN2", target_bir_lowering=False, debug=False,
                   num_devices=num_devices)
    xq = nc.dram_tensor("xq", [S, DIN], F32, kind="ExternalInput")
    xk = nc.dram_tensor("xk", [S, DIN], F32, kind="ExternalInput")
    xv = nc.dram_tensor("xv", [S, DIN], F32, kind="ExternalInput")
    wq = nc.dram_tensor("wq", [DIN, DC], F32, kind="ExternalInput")
    wk = nc.dram_tensor("wk", [DIN, DC], F32, kind="ExternalInput")
    wv = nc.dram_tensor("wv", [DIN, DC], F32, kind="ExternalInput")
    wo = nc.dram_tensor("wo", [DC, DOUT], F32, kind="ExternalInput")
    bq = nc.dram_tensor("bq", [DC], F32, kind="ExternalInput")
    bk = nc.dram_tensor("bk", [DC], F32, kind="ExternalInput")
    bv = nc.dram_tensor("bv", [DC], F32, kind="ExternalInput")
    out = nc.dram_tensor("out", [S, DOUT], F32, kind="ExternalOutput")
    if dbg:
        d_xt = nc.dram_tensor("d_xt", [P, 8 * 512], F32, kind="ExternalOutput")
        d_kt = nc.dram_tensor("d_kt", [P, 4 * 512], F32, kind="ExternalOutput")
        d_v = nc.dram_tensor("d_v", [P, 8 * 65], F32, kind="ExternalOutput")
        d_qt = nc.dram_tensor("d_qt", [P, 4 * 512], F32, kind="ExternalOutput")
        d_st = nc.dram_tensor("d_st", [P, 2 * 512], F32, kind="ExternalOutput")
        d_ex = nc.dram_tensor("d_ex", [P, 2 * 512], F32, kind="ExternalOutput")
        d_ot = nc.dram_tensor("d_ot", [P, 512], F32, kind="ExternalOutput")
        d_on = nc.dram_tensor("d_on", [P, 4 * 8 * 64], F32, kind="ExternalOutput")
        d_otn = nc.dram_tensor("d_otn", [P, 4 * 512], F32, kind="ExternalOutput")
        d_kt2 = nc.dram_tensor("d_kt2", [P, 4 * 512], F32, kind="ExternalOutput")
        d_v2 = nc.dram_tensor("d_v2", [P, 8 * 65], F32, kind="ExternalOutput")
        d_xt2 = nc.dram_tensor("d_xt2", [P, 8 * 512], F32, kind="ExternalOutput")

    with TileContext(nc) as tc, ExitStack() as ctx:
        const = ctx.enter_context(tc.tile_pool(name="const", bufs=1))
        wpool = ctx.enter_context(tc.tile_pool(name="wpool", bufs=1))
        kvpool = ctx.enter_context(tc.tile_pool(name="kv", bufs=1))
        xnpool = ctx.enter_context(tc.tile_pool(name="xn", bufs=4))
        xtpool = ctx.enter_context(tc.tile_pool(name="xt", bufs=4))
        xqnpool = ctx.enter_context(tc.tile_pool(name="xqn", bufs=2))
        xqtpool = ctx.enter_context(tc.tile_pool(name="xqt", bufs=2))

        # ---- weights: casting DMA f32 -> bf16, split into <=4KB/partition
        def load_weight(dram, kdim, ndim, name):
            w = wpool.tile([P, kdim // P, ndim], BF16, name=name)
            nc.gpsimd.dma_start(
                w[:], dram[:, :].rearrange("(o p) n -> p o n", p=P))
            return w

        # ---- x loading: casting DMA to bf16 natural layout, then XBAR
        # transpose chunks of 128 rows into [din_part, NDIN, s] layout.
        def load_xn(xdram, r0, nrows, tag, npool=None):
            npool = npool or xnpool
            nch = nrows // P
            xn = npool.tile([P, nch, DIN], BF16, tag="xn", name="xn" + tag)
            nc.gpsimd.dma_start(
                xn[:],
                xdram[r0:r0 + nrows, :].rearrange("(c p) d -> p c d", p=P))
            return xn

        def transpose_xn(xn, nrows, tag, tpool=None):
            tpool = tpool or xtpool
            xt = tpool.tile([P, NDIN, nrows], BF16, tag="xt", name="xt" + tag)
            for c in range(nrows // P):
                nc.sync.dma_start_transpose(
                    xt[:, :, c * P:(c + 1) * P], xn[:, c, :])
            return xt

        def load_xt(xdram, r0, nrows, tag, npool=None, tpool=None):
            xn = load_xn(xdram, r0, nrows, tag, npool)
            return transpose_xn(xn, nrows, tag, tpool)

        # ---- persistent K^T and V ----
        KT = kvpool.tile([P, NDO, S], BF16)
        V = kvpool.tile([P, NKT, H, depth + 1], BF16)
        nc.vector.memset(V[:, :, :, depth:depth + 1], 1.0)

        if dbg:
            dbgpool = ctx.enter_context(tc.tile_pool(name="dbgp", bufs=2))

        def dump(dram, src):
            n = 1
            for d in src.shape[1:]:
                n *= d
            stg = dbgpool.tile([P] + list(src.shape[1:]), F32, tag="dbgs",
                               name="dbgs")
            nc.vector.tensor_copy(stg[:src.shape[0]], src[:])
            nc.sync.dma_start(
                dram[0:src.shape[0], 0:n],
                stg[:src.shape[0]].rearrange(
                    {2: "p a -> p a", 3: "p a b -> p (a b)",
                     4: "p a b c -> p (a b c)"}[len(src.shape)]))

        # ---- attention-phase pools ----
        qpool = ctx.enter_context(tc.tile_pool(name="qp", bufs=2))
        otnpool = ctx.enter_context(tc.tile_pool(name="otn", bufs=2))
        expool = ctx.enter_context(tc.tile_pool(name="ex", bufs=ex_bufs))
        recpool = ctx.enter_context(tc.tile_pool(name="rec", bufs=2))
        osbpool = ctx.enter_context(tc.tile_pool(name="osb", bufs=2))
        ps_st = ctx.enter_context(
            tc.tile_pool(name="ps_st", bufs=st_bufs, space="PSUM"))
        ps_ot = ctx.enter_context(
            tc.tile_pool(name="ps_ot", bufs=ot_bufs, space="PSUM"))
        ps_gen = ctx.enter_context(
            tc.tile_pool(name="ps_gen", bufs=gen_bufs, space="PSUM"))

        def attn_head(QT, OTn, h, dump_first=False, kt_filler=None):
            blk, p0 = h // 2, (h % 2) * 64
            ot = ps_ot.tile([depth + 1, SQT], F32, tag="ot", name="ot")
            pend = []  # software pipeline: (ex, kg) waiting for OT emission

            def emit_ot(ex, kg):
                for j in range(KG):
                    kt = kg * KG + j
                    nc.tensor.matmul(
                        ot[:], V[:, kt, h, :], ex[:, j, :],
                        start=(kt == 0), stop=(kt == NKT - 1))

            for kg in range(NKG):
                st = ps_st.tile([P, KG, 512], F32, tag="st", name="st")
                for j in range(KG):
                    kt = kg * KG + j
                    nc.tensor.matmul(
                        st[:, j], KT[p0:p0 + 64, blk, kt * P:(kt + 1) * P],
                        QT[p0:p0 + 64, blk, :], start=True, stop=True)
                ex = expool.tile([P, KG, 512], BF16, tag="ex", name="ex")
                if QA > 0:
                    nc.scalar.activation(ex[:, :, 0:QA], st[:, :, 0:QA],
                                         EXP, scale=scale)
                if QA < 512:
                    nc.vector.tensor_scalar(
                        ex[:, :, QA:512].bitcast(I16), st[:, :, QA:512],
                        a_exp, b_exp,
                        mybir.AluOpType.mult, mybir.AluOpType.add)
                if dump_first and kg == 3:
                    dump(d_st, st[:])
                    dump(d_ex, ex[:])
                if kt_filler is not None:
                    kt_filler(kg)
                pend.append((ex, kg))
                if len(pend) >= pdepth:
                    emit_ot(*pend.pop(0))
            for pe in pend:
                emit_ot(*pe)
            if dump_first:
                dump(d_ot, ot[:])

            # normalize: ot row 64 is the softmax denominator.  Compute at
            # partition base 0; plain tensor_copy shifts bases (HW-proven).
            den = recpool.tile([1, SQT], F32, tag="den", name="den")
            nc.vector.tensor_copy(den[0:1, :], ot[depth:depth + 1, :])
            rec = recpool.tile([1, SQT], F32, tag="rec", name="rec")
            nc.vector.reciprocal(rec[0:1, :], den[0:1, :])
            bc = recpool.tile([64, SQT], F32, tag="bc", name="bc")
            nc.gpsimd.partition_broadcast(bc[0:64, :], rec[0:1, :])
            onorm = recpool.tile([64, SQT], BF16, tag="onorm", name="onorm")
            nc.vector.tensor_tensor(onorm[0:64, :], ot[0:depth, :],
                                    bc[0:64, :], mybir.AluOpType.mult)
            nc.vector.tensor_copy(OTn[p0:p0 + 64, blk, :], onorm[0:64, :])

        # ---- main attention loop over q tiles ----

        def qproj_start(sqt):
            """DMA work for Q tile sqt: load + transpose; returns (xt, QT)."""
            xt = load_xt(xq, sqt * SQT, SQT, "q", npool=xqnpool, tpool=xqtpool)
            QT = qpool.tile([P, NDO, SQT], BF16, tag="qt", name="qt")
            return xt, QT

        def qproj_group(xt, QT, do):
            ps = ps_gen.tile([P, 512], F32, tag="gen", name="psq")
            psq = ps[:, :SQT]
            for kt in range(NDIN):
                nc.tensor.matmul(
                    psq[:], wq_sb[:, kt, do * P:(do + 1) * P], xt[:, kt, :],
                    start=(kt == 0), stop=(kt == NDIN - 1))
            nc.vector.tensor_scalar_add(QT[:, do, :], psq[:],
                                        bq_sb[:, do:do + 1])

        def outproj_group(OTn, sqt, g):
            do, sc = g // NQC, g % NQC
            ps = ps_gen.tile([P, 512], F32, tag="gen", name="pso")
            for hh in range(NDO):
                nc.tensor.matmul(
                    ps[:], OTn[:, hh, sc * P:(sc + 1) * P],
                    wo_sb[:, hh, do * 512:(do + 1) * 512],
                    start=(hh == 0), stop=(hh == NDO - 1))
            osb = osbpool.tile([P, 512], F32, tag="osb", name="osb")
            nc.scalar.activation(osb[:], ps[:], COPYF)
            r0 = sqt * SQT + sc * P
            nc.sync.dma_start(out[r0:r0 + P, do * 512:(do + 1) * 512], osb[:])

        if True:
            # K projection: xk load trains are issued up front so the casting
            # DMAs and XBAR transposes stream back-to-back; weight and xq
            # loads follow in consumption order.
            wk_sb = load_weight(wk, DIN, DC, "wk_sb")
            bk_sb = const.tile([P, NDO], F32)
            nc.sync.dma_start(bk_sb[:], bk[:].rearrange("(o p) -> p o", p=P))
            bq_sb = const.tile([P, NDO], F32)
            nc.sync.dma_start(bq_sb[:], bq[:].rearrange("(o p) -> p o", p=P))
            bv_st = const.tile([1, DC], F32)
            nc.sync.dma_start(bv_st[0:1, :], bv[:][None, :])
            bv_bc = const.tile([P, DC], F32)
            nc.gpsimd.partition_broadcast(bv_bc[:], bv_st[0:1, :])
            NST = S // 512
            xkn = [load_xn(xk, 0, 512, "k"), load_xn(xk, 512, 512, "k")]
            xkt = []
            for i in range(NST):
                if i + 2 < NST:
                    xkn.append(load_xn(xk, (i + 2) * 512, 512, "k"))
                xkt.append(transpose_xn(xkn[i], 512, "k"))
            for st_i in range(NST):
                xt = xkt[st_i]
                if st_i == 0:
                    wq_sb = load_weight(wq, DIN, DC, "wq_sb")
                    xn_q0 = load_xn(xq, 0, SQT, "q", npool=xqnpool)
                elif st_i == 1:
                    xt_q0 = transpose_xn(xn_q0, SQT, "q", tpool=xqtpool)
                elif st_i == 2:
                    wv_sb = load_weight(wv, DIN, DC, "wv_sb")
                elif st_i == 3:
                    xvn_pre = [load_xn(xv, 0, 512, "v"),
                               load_xn(xv, 512, 512, "v")]
                if dbg and st_i == 0:
                    dump(d_xt, xt[:])
                if dbg and st_i == 1:
                    dump(d_xt2, xt[:])
                for do in range(NDO):
                    ps = ps_gen.tile([P, 512], F32, tag="gen", name="psk")
                    for kt in range(NDIN):
                        nc.tensor.matmul(
                            ps[:], wk_sb[:, kt, do * P:(do + 1) * P],
                            xt[:, kt, :],
                            start=(kt == 0), stop=(kt == NDIN - 1))
                    nc.scalar.activation(
                        KT[:, do, st_i * 512:(st_i + 1) * 512], ps[:],
                        IDENT, bias=bk_sb[:, do:do + 1])

            # V projection is deferred: chunk c is produced as a PE filler
            # inside attention head 0 of sqt 0, just ahead of the OT matmul
            # that consumes it; xv load trains are issued before it starts.
            QT0 = qpool.tile([P, NDO, SQT], BF16, tag="qt", name="qt")
            for do in range(NDO):
                qproj_group(xt_q0, QT0, do)
            xvn = xvn_pre
            xvt_tiles = {}
            for i in range(S // 512):
                if i + 2 < S // 512:
                    xvn.append(load_xn(xv, (i + 2) * 512, 512, "v"))
                xvt_tiles[i] = transpose_xn(xvn[i], 512, "v")

            def vchunk_group(c):
                st_i, sc = c // 4, c % 4
                xtv = xvt_tiles[st_i]
                ps = ps_gen.tile([P, 512], F32, tag="gen", name="psv")
                psd = ps[:, :DC]
                for kt in range(NDIN):
                    nc.tensor.matmul(
                        psd[:], xtv[:, kt, sc * P:(sc + 1) * P],
                        wv_sb[:, kt, :],
                        start=(kt == 0), stop=(kt == NDIN - 1))
                nc.vector.tensor_tensor(
                    V[:, c, :, 0:depth],
                    psd[:].rearrange("p (h d) -> p h d", h=H),
                    bv_bc[:].rearrange("p (h d) -> p h d", h=H),
                    mybir.AluOpType.add)

        QT_cur = QT0
        if dbg:
            dump(d_kt, KT[:, :, 0:512])
            dump(d_v, V[:, 0, :, :])
            dump(d_kt2, KT[:, :, 512:1024])
            dump(d_v2, V[:, 5, :, :])
            dump(d_qt, QT_cur[:])

        OTn_prev = None
        for sqt in range(NSQT):
            OTn = otnpool.tile([P, NDO, SQT], BF16, tag="otn", name="otn")
            if sqt + 1 < NSQT:
                xt_q, QT_next = qproj_start(sqt + 1)
            else:
                QT_next = None

            for h in range(H):
                attn_head(QT_cur, OTn, h,
                          dump_first=(dbg and sqt == 0 and h == 0),
                          kt_filler=(vchunk_group if sqt == 0 and h == 0
                                     else None))
                if sqt == 0 and h == 2:
                    wo_sb = load_weight(wo, DC, DOUT, "wo_sb")
                # interleave projection fillers between heads
                if h % 2 == 0 and QT_next is not None:
                    qproj_group(xt_q, QT_next, h // 2)
                elif OTn_prev is not None and h % 2 == 1:
                    outproj_group(OTn_prev, sqt - 1, h - 1)
                    outproj_group(OTn_prev, sqt - 1, h)
            if dbg and sqt == 0:
                dump(d_otn, OTn[:])

            OTn_prev, QT_cur = OTn, QT_next

        # final sqt's out-projection
        for g in range(2 * NQC):
            outproj_group(OTn_prev, NSQT - 1, g)

    nc.compile()
    return nc


# ---------------------------------------------------------------------------
# Host-side wrapper: shard across 8 NeuronCores, run SPMD, gather.
# Core c handles batch b = c // 2 and head-group g = c % 2 (8 of 16 heads,
# i.e. columns [g*512, (g+1)*512) of Wq/Wk/Wv and rows of Wo).
# ---------------------------------------------------------------------------

import numpy as np

from concourse.bass_utils import run_bass_kernel_spmd

_NC = None


def _get_nc():
    global _NC
    if _NC is None:
        _NC = build_mha_core(S=2048, DIN=1024, DC=512, DOUT=1024, H=8,
                             depth=64, num_devices=8)
    return _NC


def _in_maps(q, k, v, Wq, bq, Wk, bk, Wv, bv, Wo, bo):
    f32 = np.float32
    maps = []
    for c in range(8):
        b, g = c // 2, c % 2
        sl = slice(g * 512, (g + 1) * 512)
        maps.append({
            "xq": np.ascontiguousarray(q[b], dtype=f32),
            "xk": np.ascontiguousarray(k[b], dtype=f32),
            "xv": np.ascontiguousarray(v[b], dtype=f32),
            "wq": np.ascontiguousarray(Wq[:, sl], dtype=f32),
            "wk": np.ascontiguousarray(Wk[:, sl], dtype=f32),
            "wv": np.ascontiguousarray(Wv[:, sl], dtype=f32),
            "wo": np.ascontiguousarray(Wo[sl, :], dtype=f32),
            "bq": np.ascontiguousarray(bq[sl], dtype=f32),
            "bk": np.ascontiguousarray(bk[sl], dtype=f32),
            "bv": np.ascontiguousarray(bv[sl], dtype=f32),
        })
    return maps


def _gather(results, bo):
    out = np.empty((4, 2048, 1024), dtype=np.float32)
    bo32 = np.asarray(bo, dtype=np.float32)
    for b in range(4):
        out[b] = results[2 * b]["out"] + results[2 * b + 1]["out"] + bo32
    return out


def kernel(q, k, v, Wq, bq, Wk, bk, Wv, bv, Wo, bo, _trace=False):
    nc = _get_nc()
    res = run_bass_kernel_spmd(
        nc, _in_maps(q, k, v, Wq, bq, Wk, bk, Wv, bv, Wo, bo),
        core_ids=list(range(8)), trace=_trace)
    out = _gather(res.results, bo)
    if _trace:
        kernel.last_results = res
    return out


# revision 51
# speedup vs baseline: 1.1230x; 1.1230x over previous
"""Bass/Tile multi-head attention kernel for TRN2 (v2, all-bf16 datapath).

Per-core problem (core c handles batch b=c//2, head-group g=c%2):
  inputs:  xq, xk, xv [S, DIN] f32      (batch b slices of q/k/v)
           wq, wk, wv [DIN, DC] f32     (column slice for this head group)
           wo [DC, DOUT] f32            (row slice)
           bq, bk, bv [DC] f32
  output:  out [S, DOUT] f32  partial:  host sums the two head-group partials
           per batch and adds bo.

Key structure (H=8 local heads, depth=64, P=128):
  - x / weights enter SBUF as bf16 via gpsimd casting DMAs; x is transposed
    by the DMA XBAR (dma_start_transpose), so the PE does no transposes.
  - KT/QT are packed 2-heads-per-128-partition blocks; per-head matmuls use
    64-partition views at base 0/64 (contraction over depth=64).
  - ST:  st[k, q] = KT_h^T @ QT_h        (N=512 moving)
  - exp: split between ACT (exact, cols 0:QA) and DVE (Schraudolph int16
    bit-trick -> bf16 bits, cols QA:512), software-pipelined pdepth deep.
  - OT:  ot[d+1, q] += V_aug_h^T @ ex    (V stationary; row 64 of V_aug is
         ones -> softmax denominator in ot[64, :])
  - normalize: reciprocal + partition_broadcast + multiply, written into
    OTn [dc, q] bf16 (plain tensor_copy shifts partition base, HW-proven).
  - V-projection chunks are deferred into attention head 0 of sqt 0 as PE
    fillers; qproj(sqt+1)/outproj(sqt-1) fill between later heads.
"""

from contextlib import ExitStack

import concourse.mybir as mybir
from concourse import bacc
from concourse.tile import TileContext

F32 = mybir.dt.float32
BF16 = mybir.dt.bfloat16
I16 = mybir.dt.int16
P = 128
EXP = mybir.ActivationFunctionType.Exp
IDENT = mybir.ActivationFunctionType.Identity
COPYF = mybir.ActivationFunctionType.Copy
LOG2E = 1.4426950408889634


def build_mha_core(S=2048, DIN=1024, DC=512, DOUT=1024, H=8, depth=64,
                   SQT=512, KG=1, QA=352, num_devices=1,
                   st_bufs=4, ex_bufs=6, ot_bufs=2, gen_bufs=2, pdepth=5,
                   dbg=False):
    assert DC == H * depth and DC % P == 0 and DIN % P == 0 and S % SQT == 0
    NKT = S // P          # key chunks of 128
    NDIN = DIN // P       # input-dim k-tiles
    NDO = DC // P         # d_core blocks (2 heads each)
    NSQT = S // SQT       # attention q tiles
    NQC = SQT // P        # 128-wide q chunks per sqt
    NKG = NKT // KG
    scale = 1.0 / float(depth) ** 0.5
    a_exp = 128.0 * LOG2E * scale
    b_exp = 16250.4

    nc = bacc.Bacc("TRN2", target_bir_lowering=False, debug=False,
                   num_devices=num_devices)
    xq = nc.dram_tensor("xq", [S, DIN], F32, kind="ExternalInput")
    xk = nc.dram_tensor("xk", [S, DIN], F32, kind="ExternalInput")
    xv = nc.dram_tensor("xv", [S, DIN], F32, kind="ExternalInput")
    wq = nc.dram_tensor("wq", [DIN, DC], F32, kind="ExternalInput")
    wk = nc.dram_tensor("wk", [DIN, DC], F32, kind="ExternalInput")
    wv = nc.dram_tensor("wv", [DIN, DC], F32, kind="ExternalInput")
    wo = nc.dram_tensor("wo", [DC, DOUT], F32, kind="ExternalInput")
    bq = nc.dram_tensor("bq", [DC], F32, kind="ExternalInput")
    bk = nc.dram_tensor("bk", [DC], F32, kind="ExternalInput")
    bv = nc.dram_tensor("bv", [DC], F32, kind="ExternalInput")
    out = nc.dram_tensor("out", [S, DOUT], F32, kind="ExternalOutput")
    if dbg:
        d_xt = nc.dram_tensor("d_xt", [P, 8 * 512], F32, kind="ExternalOutput")
        d_kt = nc.dram_tensor("d_kt", [P, 4 * 512], F32, kind="ExternalOutput")
        d_v = nc.dram_tensor("d_v", [P, 8 * 65], F32, kind="ExternalOutput")
        d_qt = nc.dram_tensor("d_qt", [P, 4 * 512], F32, kind="ExternalOutput")
        d_st = nc.dram_tensor("d_st", [P, 2 * 512], F32, kind="ExternalOutput")
        d_ex = nc.dram_tensor("d_ex", [P, 2 * 512], F32, kind="ExternalOutput")
        d_ot = nc.dram_tensor("d_ot", [P, 512], F32, kind="ExternalOutput")
        d_on = nc.dram_tensor("d_on", [P, 4 * 8 * 64], F32, kind="ExternalOutput")
        d_otn = nc.dram_tensor("d_otn", [P, 4 * 512], F32, kind="ExternalOutput")
        d_kt2 = nc.dram_tensor("d_kt2", [P, 4 * 512], F32, kind="ExternalOutput")
        d_v2 = nc.dram_tensor("d_v2", [P, 8 * 65], F32, kind="ExternalOutput")
        d_xt2 = nc.dram_tensor("d_xt2", [P, 8 * 512], F32, kind="ExternalOutput")

    with TileContext(nc) as tc, ExitStack() as ctx:
        const = ctx.enter_context(tc.tile_pool(name="const", bufs=1))
        wpool = ctx.enter_context(tc.tile_pool(name="wpool", bufs=1))
        kvpool = ctx.enter_context(tc.tile_pool(name="kv", bufs=1))
        xnpool = ctx.enter_context(tc.tile_pool(name="xn", bufs=4))
        xtpool = ctx.enter_context(tc.tile_pool(name="xt", bufs=4))
        xqnpool = ctx.enter_context(tc.tile_pool(name="xqn", bufs=2))
        xqtpool = ctx.enter_context(tc.tile_pool(name="xqt", bufs=2))

        # ---- weights: casting DMA f32 -> bf16, split into <=4KB/partition
        def load_weight(dram, kdim, ndim, name):
            w = wpool.tile([P, kdim // P, ndim], BF16, name=name)
            nc.gpsimd.dma_start(
                w[:], dram[:, :].rearrange("(o p) n -> p o n", p=P))
            return w

        # ---- x loading: casting DMA to bf16 natural layout, then XBAR
        # transpose chunks of 128 rows into [din_part, NDIN, s] layout.
        def load_xn(xdram, r0, nrows, tag, npool=None):
            npool = npool or xnpool
            nch = nrows // P
            xn = npool.tile([P, nch, DIN], BF16, tag="xn", name="xn" + tag)
            nc.gpsimd.dma_start(
                xn[:],
                xdram[r0:r0 + nrows, :].rearrange("(c p) d -> p c d", p=P))
            return xn

        def transpose_xn(xn, nrows, tag, tpool=None):
            tpool = tpool or xtpool
            xt = tpool.tile([P, NDIN, nrows], BF16, tag="xt", name="xt" + tag)
            for c in range(nrows // P):
                nc.sync.dma_start_transpose(
                    xt[:, :, c * P:(c + 1) * P], xn[:, c, :])
            return xt

        def load_xt(xdram, r0, nrows, tag, npool=None, tpool=None):
            xn = load_xn(xdram, r0, nrows, tag, npool)
            return transpose_xn(xn, nrows, tag, tpool)

        def transpose_xn_pe(xn, nrows, tag, psum_pool, tpool=None):
            # PE-side transpose: 53ns per 128x128 bf16 tile, frees the DMA
            # engines during the front phase.  PSUM f32 -> bf16 copy on DVE.
            tpool = tpool or xtpool
            nch = nrows // P
            xt = tpool.tile([P, NDIN, nrows], BF16, tag="xt", name="xt" + tag)
            for dblk in range(NDIN):
                tp = psum_pool.tile([P, KG, 512], F32, tag="st", name="tpx")
                tpb = tp[:, 0, 0:nrows // 2].bitcast(BF16)
                for c in range(nch):
                    nc.tensor.transpose(
                        tpb[:, c * P:(c + 1) * P],
                        xn[:, c, dblk * P:(dblk + 1) * P], ident[:])
                nc.vector.tensor_copy(xt[:, dblk, :], tpb[:])
            return xt

        from concourse.masks import make_identity
        ident = const.tile([P, P], BF16)
        make_identity(nc, ident)

        # ---- persistent K^T and V ----
        KT = kvpool.tile([P, NDO, S], BF16)
        V = kvpool.tile([P, NKT, H, depth + 1], BF16)
        nc.vector.memset(V[:, :, :, depth:depth + 1], 1.0)

        if dbg:
            dbgpool = ctx.enter_context(tc.tile_pool(name="dbgp", bufs=2))

        def dump(dram, src):
            n = 1
            for d in src.shape[1:]:
                n *= d
            stg = dbgpool.tile([P] + list(src.shape[1:]), F32, tag="dbgs",
                               name="dbgs")
            nc.vector.tensor_copy(stg[:src.shape[0]], src[:])
            nc.sync.dma_start(
                dram[0:src.shape[0], 0:n],
                stg[:src.shape[0]].rearrange(
                    {2: "p a -> p a", 3: "p a b -> p (a b)",
                     4: "p a b c -> p (a b c)"}[len(src.shape)]))

        # ---- attention-phase pools ----
        qpool = ctx.enter_context(tc.tile_pool(name="qp", bufs=2))
        otnpool = ctx.enter_context(tc.tile_pool(name="otn", bufs=2))
        expool = ctx.enter_context(tc.tile_pool(name="ex", bufs=ex_bufs))
        recpool = ctx.enter_context(tc.tile_pool(name="rec", bufs=2))
        osbpool = ctx.enter_context(tc.tile_pool(name="osb", bufs=2))
        ps_st = ctx.enter_context(
            tc.tile_pool(name="ps_st", bufs=st_bufs, space="PSUM"))
        ps_ot = ctx.enter_context(
            tc.tile_pool(name="ps_ot", bufs=ot_bufs, space="PSUM"))
        ps_gen = ctx.enter_context(
            tc.tile_pool(name="ps_gen", bufs=gen_bufs, space="PSUM"))

        def attn_head(QT, OTn, h, dump_first=False, kt_filler=None,
                      norm_filler=None):
            blk, p0 = h // 2, (h % 2) * 64
            ot = ps_ot.tile([depth + 1, SQT], F32, tag="ot", name="ot")
            pend = []  # software pipeline: (ex, kg) waiting for OT emission

            def emit_ot(ex, kg):
                for j in range(KG):
                    kt = kg * KG + j
                    nc.tensor.matmul(
                        ot[:], V[:, kt, h, :], ex[:, j, :],
                        start=(kt == 0), stop=(kt == NKT - 1))

            for kg in range(NKG):
                st = ps_st.tile([P, KG, 512], F32, tag="st", name="st")
                for j in range(KG):
                    kt = kg * KG + j
                    nc.tensor.matmul(
                        st[:, j], KT[p0:p0 + 64, blk, kt * P:(kt + 1) * P],
                        QT[p0:p0 + 64, blk, :], start=True, stop=True)
                ex = expool.tile([P, KG, 512], BF16, tag="ex", name="ex")
                if QA > 0:
                    nc.scalar.activation(ex[:, :, 0:QA], st[:, :, 0:QA],
                                         EXP, scale=scale)
                if QA < 512:
                    nc.vector.tensor_scalar(
                        ex[:, :, QA:512].bitcast(I16), st[:, :, QA:512],
                        a_exp, b_exp,
                        mybir.AluOpType.mult, mybir.AluOpType.add)
                if dump_first and kg == 3:
                    dump(d_st, st[:])
                    dump(d_ex, ex[:])
                if kt_filler is not None:
                    kt_filler(kg)
                if kg == 1 and norm_filler is not None:
                    # previous head's normalize runs here, behind this head's
                    # first exps, so it never delays the OT pipeline start
                    norm_filler()
                pend.append((ex, kg))
                if len(pend) >= pdepth:
                    emit_ot(*pend.pop(0))
            for pe in pend:
                emit_ot(*pe)
            if dump_first:
                dump(d_ot, ot[:])

            def do_normalize():
                # ot row 64 is the softmax denominator.  Compute at partition
                # base 0; plain tensor_copy shifts bases (HW-proven).
                den = recpool.tile([1, SQT], F32, tag="den", name="den")
                nc.vector.tensor_copy(den[0:1, :], ot[depth:depth + 1, :])
                rec = recpool.tile([1, SQT], F32, tag="rec", name="rec")
                nc.vector.reciprocal(rec[0:1, :], den[0:1, :])
                bc = recpool.tile([64, SQT], F32, tag="bc", name="bc")
                nc.gpsimd.partition_broadcast(bc[0:64, :], rec[0:1, :])
                onorm = recpool.tile([64, SQT], BF16, tag="onorm",
                                     name="onorm")
                nc.vector.tensor_tensor(onorm[0:64, :], ot[0:depth, :],
                                        bc[0:64, :], mybir.AluOpType.mult)
                nc.vector.tensor_copy(OTn[p0:p0 + 64, blk, :], onorm[0:64, :])
            return do_normalize

        # ---- main attention loop over q tiles ----

        def qproj_start(sqt):
            """DMA work for Q tile sqt: load + transpose; returns (xt, QT)."""
            xt = load_xt(xq, sqt * SQT, SQT, "q", npool=xqnpool, tpool=xqtpool)
            QT = qpool.tile([P, NDO, SQT], BF16, tag="qt", name="qt")
            return xt, QT

        def qproj_group(xt, QT, do):
            ps = ps_gen.tile([P, 512], F32, tag="gen", name="psq")
            psq = ps[:, :SQT]
            for kt in range(NDIN):
                nc.tensor.matmul(
                    psq[:], wq_sb[:, kt, do * P:(do + 1) * P], xt[:, kt, :],
                    start=(kt == 0), stop=(kt == NDIN - 1))
            nc.vector.tensor_scalar_add(QT[:, do, :], psq[:],
                                        bq_sb[:, do:do + 1])

        def outproj_group(OTn, sqt, g):
            do, sc = g // NQC, g % NQC
            ps = ps_gen.tile([P, 512], F32, tag="gen", name="pso")
            for hh in range(NDO):
                nc.tensor.matmul(
                    ps[:], OTn[:, hh, sc * P:(sc + 1) * P],
                    wo_sb[:, hh, do * 512:(do + 1) * 512],
                    start=(hh == 0), stop=(hh == NDO - 1))
            osb = osbpool.tile([P, 512], F32, tag="osb", name="osb")
            nc.scalar.activation(osb[:], ps[:], COPYF)
            r0 = sqt * SQT + sc * P
            nc.sync.dma_start(out[r0:r0 + P, do * 512:(do + 1) * 512], osb[:])

        if True:
            # K projection: xk load trains are issued up front so the casting
            # DMAs and XBAR transposes stream back-to-back; weight and xq
            # loads follow in consumption order.
            wk_sb = load_weight(wk, DIN, DC, "wk_sb")
            bk_sb = const.tile([P, NDO], F32)
            nc.sync.dma_start(bk_sb[:], bk[:].rearrange("(o p) -> p o", p=P))
            bq_sb = const.tile([P, NDO], F32)
            nc.sync.dma_start(bq_sb[:], bq[:].rearrange("(o p) -> p o", p=P))
            bv_st = const.tile([1, DC], F32)
            nc.sync.dma_start(bv_st[0:1, :], bv[:][None, :])
            bv_bc = const.tile([P, DC], F32)
            nc.gpsimd.partition_broadcast(bv_bc[:], bv_st[0:1, :])
            NST = S // 512
            xkn = [load_xn(xk, i * 512, 512, "k") for i in range(NST)]
            xkt = [transpose_xn_pe(xkn[i], 512, "k", ps_st)
                   for i in range(NST)]
            for st_i in range(NST):
                xt = xkt[st_i]
                if st_i == 0:
                    wq_sb = load_weight(wq, DIN, DC, "wq_sb")
                    xn_q0 = load_xn(xq, 0, SQT, "q", npool=xqnpool)
                elif st_i == 1:
                    xt_q0 = transpose_xn_pe(xn_q0, SQT, "q", ps_st,
                                            tpool=xqtpool)
                elif st_i == 2:
                    wv_sb = load_weight(wv, DIN, DC, "wv_sb")
                elif st_i == 3:
                    xvn_pre = [load_xn(xv, 0, 512, "v"),
                               load_xn(xv, 512, 512, "v")]
                if dbg and st_i == 0:
                    dump(d_xt, xt[:])
                if dbg and st_i == 1:
                    dump(d_xt2, xt[:])
                for do in range(NDO):
                    ps = ps_gen.tile([P, 512], F32, tag="gen", name="psk")
                    for kt in range(NDIN):
                        nc.tensor.matmul(
                            ps[:], wk_sb[:, kt, do * P:(do + 1) * P],
                            xt[:, kt, :],
                            start=(kt == 0), stop=(kt == NDIN - 1))
                    nc.scalar.activation(
                        KT[:, do, st_i * 512:(st_i + 1) * 512], ps[:],
                        IDENT, bias=bk_sb[:, do:do + 1])

            # V projection is deferred: chunk c is produced as a PE filler
            # inside attention head 0 of sqt 0, just ahead of the OT matmul
            # that consumes it; xv load trains are issued before it starts.
            QT0 = qpool.tile([P, NDO, SQT], BF16, tag="qt", name="qt")
            for do in range(NDO):
                qproj_group(xt_q0, QT0, do)
            xvn = xvn_pre
            xvt_tiles = {}
            for i in range(S // 512):
                if i + 2 < S // 512:
                    xvn.append(load_xn(xv, (i + 2) * 512, 512, "v"))
                xvt_tiles[i] = transpose_xn_pe(xvn[i], 512, "v", ps_st)
            del xvn

            def vchunk_group(c):
                st_i, sc = c // 4, c % 4
                xtv = xvt_tiles[st_i]
                ps = ps_gen.tile([P, 512], F32, tag="gen", name="psv")
                psd = ps[:, :DC]
                for kt in range(NDIN):
                    nc.tensor.matmul(
                        psd[:], xtv[:, kt, sc * P:(sc + 1) * P],
                        wv_sb[:, kt, :],
                        start=(kt == 0), stop=(kt == NDIN - 1))
                nc.vector.tensor_tensor(
                    V[:, c, :, 0:depth],
                    psd[:].rearrange("p (h d) -> p h d", h=H),
                    bv_bc[:].rearrange("p (h d) -> p h d", h=H),
                    mybir.AluOpType.add)

        QT_cur = QT0
        if dbg:
            dump(d_kt, KT[:, :, 0:512])
            dump(d_v, V[:, 0, :, :])
            dump(d_kt2, KT[:, :, 512:1024])
            dump(d_v2, V[:, 5, :, :])
            dump(d_qt, QT_cur[:])

        OTn_prev = None
        pend_norm = None
        for sqt in range(NSQT):
            OTn = otnpool.tile([P, NDO, SQT], BF16, tag="otn", name="otn")
            if sqt + 1 < NSQT:
                xt_q, QT_next = qproj_start(sqt + 1)
            else:
                QT_next = None

            for h in range(H):
                pend_norm = attn_head(
                    QT_cur, OTn, h,
                    dump_first=(dbg and sqt == 0 and h == 0),
                    kt_filler=(vchunk_group if sqt == 0 and h == 0
                               else None),
                    norm_filler=pend_norm)
                if sqt == 0 and h == 2:
                    wo_sb = load_weight(wo, DC, DOUT, "wo_sb")
                # interleave projection fillers between heads
                if h % 2 == 0 and QT_next is not None:
                    qproj_group(xt_q, QT_next, h // 2)
                elif OTn_prev is not None and h % 2 == 1:
                    outproj_group(OTn_prev, sqt - 1, h - 1)
                    outproj_group(OTn_prev, sqt - 1, h)
            if dbg and sqt == 0:
                dump(d_otn, OTn[:])

            OTn_prev, QT_cur = OTn, QT_next

        pend_norm()
        # final sqt's out-projection
        for g in range(2 * NQC):
            outproj_group(OTn_prev, NSQT - 1, g)

    nc.compile()
    return nc


# ---------------------------------------------------------------------------
# Host-side wrapper: shard across 8 NeuronCores, run SPMD, gather.
# Core c handles batch b = c // 2 and head-group g = c % 2 (8 of 16 heads,
# i.e. columns [g*512, (g+1)*512) of Wq/Wk/Wv and rows of Wo).
# ---------------------------------------------------------------------------

import numpy as np

from concourse.bass_utils import run_bass_kernel_spmd

_NC = None


def _get_nc():
    global _NC
    if _NC is None:
        _NC = build_mha_core(S=2048, DIN=1024, DC=512, DOUT=1024, H=8,
                             depth=64, num_devices=8)
    return _NC


def _in_maps(q, k, v, Wq, bq, Wk, bk, Wv, bv, Wo, bo):
    f32 = np.float32
    maps = []
    for c in range(8):
        b, g = c // 2, c % 2
        sl = slice(g * 512, (g + 1) * 512)
        maps.append({
            "xq": np.ascontiguousarray(q[b], dtype=f32),
            "xk": np.ascontiguousarray(k[b], dtype=f32),
            "xv": np.ascontiguousarray(v[b], dtype=f32),
            "wq": np.ascontiguousarray(Wq[:, sl], dtype=f32),
            "wk": np.ascontiguousarray(Wk[:, sl], dtype=f32),
            "wv": np.ascontiguousarray(Wv[:, sl], dtype=f32),
            "wo": np.ascontiguousarray(Wo[sl, :], dtype=f32),
            "bq": np.ascontiguousarray(bq[sl], dtype=f32),
            "bk": np.ascontiguousarray(bk[sl], dtype=f32),
            "bv": np.ascontiguousarray(bv[sl], dtype=f32),
        })
    return maps


def _gather(results, bo):
    out = np.empty((4, 2048, 1024), dtype=np.float32)
    bo32 = np.asarray(bo, dtype=np.float32)
    for b in range(4):
        out[b] = results[2 * b]["out"] + results[2 * b + 1]["out"] + bo32
    return out


def kernel(q, k, v, Wq, bq, Wk, bk, Wv, bv, Wo, bo, _trace=False):
    nc = _get_nc()
    res = run_bass_kernel_spmd(
        nc, _in_maps(q, k, v, Wq, bq, Wk, bk, Wv, bv, Wo, bo),
        core_ids=list(range(8)), trace=_trace)
    out = _gather(res.results, bo)
    if _trace:
        kernel.last_results = res
    return out


# revision 52
# speedup vs baseline: 1.1235x; 1.0005x over previous
"""Bass/Tile multi-head attention kernel for TRN2 (v2, all-bf16 datapath).

Per-core problem (core c handles batch b=c//2, head-group g=c%2):
  inputs:  xq, xk, xv [S, DIN] f32      (batch b slices of q/k/v)
           wq, wk, wv [DIN, DC] f32     (column slice for this head group)
           wo [DC, DOUT] f32            (row slice)
           bq, bk, bv [DC] f32
  output:  out [S, DOUT] f32  partial:  host sums the two head-group partials
           per batch and adds bo.

Key structure (H=8 local heads, depth=64, P=128):
  - x / weights enter SBUF as bf16 via gpsimd casting DMAs; x is transposed
    by the DMA XBAR (dma_start_transpose), so the PE does no transposes.
  - KT/QT are packed 2-heads-per-128-partition blocks; per-head matmuls use
    64-partition views at base 0/64 (contraction over depth=64).
  - ST:  st[k, q] = KT_h^T @ QT_h        (N=512 moving)
  - exp: split between ACT (exact, cols 0:QA) and DVE (Schraudolph int16
    bit-trick -> bf16 bits, cols QA:512), software-pipelined pdepth deep.
  - OT:  ot[d+1, q] += V_aug_h^T @ ex    (V stationary; row 64 of V_aug is
         ones -> softmax denominator in ot[64, :])
  - normalize: reciprocal + partition_broadcast + multiply, written into
    OTn [dc, q] bf16 (plain tensor_copy shifts partition base, HW-proven).
  - V-projection chunks are deferred into attention head 0 of sqt 0 as PE
    fillers; qproj(sqt+1)/outproj(sqt-1) fill between later heads.
"""

from contextlib import ExitStack

import concourse.mybir as mybir
from concourse import bacc
from concourse.tile import TileContext

F32 = mybir.dt.float32
BF16 = mybir.dt.bfloat16
I16 = mybir.dt.int16
P = 128
EXP = mybir.ActivationFunctionType.Exp
IDENT = mybir.ActivationFunctionType.Identity
COPYF = mybir.ActivationFunctionType.Copy
LOG2E = 1.4426950408889634


def build_mha_core(S=2048, DIN=1024, DC=512, DOUT=1024, H=8, depth=64,
                   SQT=512, KG=1, QA=352, num_devices=1,
                   st_bufs=4, ex_bufs=7, ot_bufs=2, gen_bufs=2, pdepth=6,
                   dbg=False):
    assert DC == H * depth and DC % P == 0 and DIN % P == 0 and S % SQT == 0
    NKT = S // P          # key chunks of 128
    NDIN = DIN // P       # input-dim k-tiles
    NDO = DC // P         # d_core blocks (2 heads each)
    NSQT = S // SQT       # attention q tiles
    NQC = SQT // P        # 128-wide q chunks per sqt
    NKG = NKT // KG
    scale = 1.0 / float(depth) ** 0.5
    a_exp = 128.0 * LOG2E * scale
    b_exp = 16250.4

    nc = bacc.Bacc("TRN2", target_bir_lowering=False, debug=False,
                   num_devices=num_devices)
    xq = nc.dram_tensor("xq", [S, DIN], F32, kind="ExternalInput")
    xk = nc.dram_tensor("xk", [S, DIN], F32, kind="ExternalInput")
    xv = nc.dram_tensor("xv", [S, DIN], F32, kind="ExternalInput")
    wq = nc.dram_tensor("wq", [DIN, DC], F32, kind="ExternalInput")
    wk = nc.dram_tensor("wk", [DIN, DC], F32, kind="ExternalInput")
    wv = nc.dram_tensor("wv", [DIN, DC], F32, kind="ExternalInput")
    wo = nc.dram_tensor("wo", [DC, DOUT], F32, kind="ExternalInput")
    bq = nc.dram_tensor("bq", [DC], F32, kind="ExternalInput")
    bk = nc.dram_tensor("bk", [DC], F32, kind="ExternalInput")
    bv = nc.dram_tensor("bv", [DC], F32, kind="ExternalInput")
    out = nc.dram_tensor("out", [S, DOUT], F32, kind="ExternalOutput")
    if dbg:
        d_xt = nc.dram_tensor("d_xt", [P, 8 * 512], F32, kind="ExternalOutput")
        d_kt = nc.dram_tensor("d_kt", [P, 4 * 512], F32, kind="ExternalOutput")
        d_v = nc.dram_tensor("d_v", [P, 8 * 65], F32, kind="ExternalOutput")
        d_qt = nc.dram_tensor("d_qt", [P, 4 * 512], F32, kind="ExternalOutput")
        d_st = nc.dram_tensor("d_st", [P, 2 * 512], F32, kind="ExternalOutput")
        d_ex = nc.dram_tensor("d_ex", [P, 2 * 512], F32, kind="ExternalOutput")
        d_ot = nc.dram_tensor("d_ot", [P, 512], F32, kind="ExternalOutput")
        d_on = nc.dram_tensor("d_on", [P, 4 * 8 * 64], F32, kind="ExternalOutput")
        d_otn = nc.dram_tensor("d_otn", [P, 4 * 512], F32, kind="ExternalOutput")
        d_kt2 = nc.dram_tensor("d_kt2", [P, 4 * 512], F32, kind="ExternalOutput")
        d_v2 = nc.dram_tensor("d_v2", [P, 8 * 65], F32, kind="ExternalOutput")
        d_xt2 = nc.dram_tensor("d_xt2", [P, 8 * 512], F32, kind="ExternalOutput")

    with TileContext(nc) as tc, ExitStack() as ctx:
        const = ctx.enter_context(tc.tile_pool(name="const", bufs=1))
        wpool = ctx.enter_context(tc.tile_pool(name="wpool", bufs=1))
        kvpool = ctx.enter_context(tc.tile_pool(name="kv", bufs=1))
        xnpool = ctx.enter_context(tc.tile_pool(name="xn", bufs=4))
        xtpool = ctx.enter_context(tc.tile_pool(name="xt", bufs=4))
        xqnpool = ctx.enter_context(tc.tile_pool(name="xqn", bufs=2))
        xqtpool = ctx.enter_context(tc.tile_pool(name="xqt", bufs=2))

        # ---- weights: casting DMA f32 -> bf16, split into <=4KB/partition
        def load_weight(dram, kdim, ndim, name):
            w = wpool.tile([P, kdim // P, ndim], BF16, name=name)
            nc.gpsimd.dma_start(
                w[:], dram[:, :].rearrange("(o p) n -> p o n", p=P))
            return w

        # ---- x loading: casting DMA to bf16 natural layout, then XBAR
        # transpose chunks of 128 rows into [din_part, NDIN, s] layout.
        def load_xn(xdram, r0, nrows, tag, npool=None):
            npool = npool or xnpool
            nch = nrows // P
            xn = npool.tile([P, nch, DIN], BF16, tag="xn", name="xn" + tag)
            nc.gpsimd.dma_start(
                xn[:],
                xdram[r0:r0 + nrows, :].rearrange("(c p) d -> p c d", p=P))
            return xn

        def transpose_xn(xn, nrows, tag, tpool=None):
            tpool = tpool or xtpool
            xt = tpool.tile([P, NDIN, nrows], BF16, tag="xt", name="xt" + tag)
            for c in range(nrows // P):
                nc.sync.dma_start_transpose(
                    xt[:, :, c * P:(c + 1) * P], xn[:, c, :])
            return xt

        def load_xt(xdram, r0, nrows, tag, npool=None, tpool=None):
            xn = load_xn(xdram, r0, nrows, tag, npool)
            return transpose_xn(xn, nrows, tag, tpool)

        def transpose_xn_pe(xn, nrows, tag, psum_pool, tpool=None):
            # PE-side transpose: 53ns per 128x128 bf16 tile, frees the DMA
            # engines during the front phase.  PSUM f32 -> bf16 copy on DVE.
            tpool = tpool or xtpool
            nch = nrows // P
            xt = tpool.tile([P, NDIN, nrows], BF16, tag="xt", name="xt" + tag)
            for dblk in range(NDIN):
                tp = psum_pool.tile([P, KG, 512], F32, tag="st", name="tpx")
                tpb = tp[:, 0, 0:nrows // 2].bitcast(BF16)
                for c in range(nch):
                    nc.tensor.transpose(
                        tpb[:, c * P:(c + 1) * P],
                        xn[:, c, dblk * P:(dblk + 1) * P], ident[:])
                nc.vector.tensor_copy(xt[:, dblk, :], tpb[:])
            return xt

        from concourse.masks import make_identity
        ident = const.tile([P, P], BF16)
        make_identity(nc, ident)

        # ---- persistent K^T and V ----
        KT = kvpool.tile([P, NDO, S], BF16)
        V = kvpool.tile([P, NKT, H, depth + 1], BF16)
        nc.vector.memset(V[:, :, :, depth:depth + 1], 1.0)

        if dbg:
            dbgpool = ctx.enter_context(tc.tile_pool(name="dbgp", bufs=2))

        def dump(dram, src):
            n = 1
            for d in src.shape[1:]:
                n *= d
            stg = dbgpool.tile([P] + list(src.shape[1:]), F32, tag="dbgs",
                               name="dbgs")
            nc.vector.tensor_copy(stg[:src.shape[0]], src[:])
            nc.sync.dma_start(
                dram[0:src.shape[0], 0:n],
                stg[:src.shape[0]].rearrange(
                    {2: "p a -> p a", 3: "p a b -> p (a b)",
                     4: "p a b c -> p (a b c)"}[len(src.shape)]))

        # ---- attention-phase pools ----
        qpool = ctx.enter_context(tc.tile_pool(name="qp", bufs=2))
        otnpool = ctx.enter_context(tc.tile_pool(name="otn", bufs=2))
        expool = ctx.enter_context(tc.tile_pool(name="ex", bufs=ex_bufs))
        recpool = ctx.enter_context(tc.tile_pool(name="rec", bufs=2))
        osbpool = ctx.enter_context(tc.tile_pool(name="osb", bufs=2))
        ps_st = ctx.enter_context(
            tc.tile_pool(name="ps_st", bufs=st_bufs, space="PSUM"))
        ps_ot = ctx.enter_context(
            tc.tile_pool(name="ps_ot", bufs=ot_bufs, space="PSUM"))
        ps_gen = ctx.enter_context(
            tc.tile_pool(name="ps_gen", bufs=gen_bufs, space="PSUM"))

        def attn_head(QT, OTn, h, dump_first=False, kt_filler=None,
                      norm_filler=None):
            blk, p0 = h // 2, (h % 2) * 64
            ot = ps_ot.tile([depth + 1, SQT], F32, tag="ot", name="ot")
            pend = []  # software pipeline: (ex, kg) waiting for OT emission

            def emit_ot(ex, kg):
                for j in range(KG):
                    kt = kg * KG + j
                    nc.tensor.matmul(
                        ot[:], V[:, kt, h, :], ex[:, j, :],
                        start=(kt == 0), stop=(kt == NKT - 1))

            for kg in range(NKG):
                st = ps_st.tile([P, KG, 512], F32, tag="st", name="st")
                for j in range(KG):
                    kt = kg * KG + j
                    nc.tensor.matmul(
                        st[:, j], KT[p0:p0 + 64, blk, kt * P:(kt + 1) * P],
                        QT[p0:p0 + 64, blk, :], start=True, stop=True)
                ex = expool.tile([P, KG, 512], BF16, tag="ex", name="ex")
                if QA > 0:
                    nc.scalar.activation(ex[:, :, 0:QA], st[:, :, 0:QA],
                                         EXP, scale=scale)
                if QA < 512:
                    nc.vector.tensor_scalar(
                        ex[:, :, QA:512].bitcast(I16), st[:, :, QA:512],
                        a_exp, b_exp,
                        mybir.AluOpType.mult, mybir.AluOpType.add)
                if dump_first and kg == 3:
                    dump(d_st, st[:])
                    dump(d_ex, ex[:])
                if kt_filler is not None:
                    kt_filler(kg)
                if kg == 1 and norm_filler is not None:
                    # previous head's normalize runs here, behind this head's
                    # first exps, so it never delays the OT pipeline start
                    norm_filler()
                pend.append((ex, kg))
                if len(pend) >= pdepth:
                    emit_ot(*pend.pop(0))
            for pe in pend:
                emit_ot(*pe)
            if dump_first:
                dump(d_ot, ot[:])

            def do_normalize():
                # ot row 64 is the softmax denominator.  Compute at partition
                # base 0; plain tensor_copy shifts bases (HW-proven).
                den = recpool.tile([1, SQT], F32, tag="den", name="den")
                nc.vector.tensor_copy(den[0:1, :], ot[depth:depth + 1, :])
                rec = recpool.tile([1, SQT], F32, tag="rec", name="rec")
                nc.vector.reciprocal(rec[0:1, :], den[0:1, :])
                bc = recpool.tile([64, SQT], F32, tag="bc", name="bc")
                nc.gpsimd.partition_broadcast(bc[0:64, :], rec[0:1, :])
                onorm = recpool.tile([64, SQT], BF16, tag="onorm",
                                     name="onorm")
                nc.vector.tensor_tensor(onorm[0:64, :], ot[0:depth, :],
                                        bc[0:64, :], mybir.AluOpType.mult)
                nc.vector.tensor_copy(OTn[p0:p0 + 64, blk, :], onorm[0:64, :])
            return do_normalize

        # ---- main attention loop over q tiles ----

        def qproj_start(sqt):
            """DMA work for Q tile sqt: load + transpose; returns (xt, QT)."""
            xt = load_xt(xq, sqt * SQT, SQT, "q", npool=xqnpool, tpool=xqtpool)
            QT = qpool.tile([P, NDO, SQT], BF16, tag="qt", name="qt")
            return xt, QT

        def qproj_group(xt, QT, do):
            ps = ps_gen.tile([P, 512], F32, tag="gen", name="psq")
            psq = ps[:, :SQT]
            for kt in range(NDIN):
                nc.tensor.matmul(
                    psq[:], wq_sb[:, kt, do * P:(do + 1) * P], xt[:, kt, :],
                    start=(kt == 0), stop=(kt == NDIN - 1))
            nc.vector.tensor_scalar_add(QT[:, do, :], psq[:],
                                        bq_sb[:, do:do + 1])

        def outproj_group(OTn, sqt, g):
            do, sc = g // NQC, g % NQC
            ps = ps_gen.tile([P, 512], F32, tag="gen", name="pso")
            for hh in range(NDO):
                nc.tensor.matmul(
                    ps[:], OTn[:, hh, sc * P:(sc + 1) * P],
                    wo_sb[:, hh, do * 512:(do + 1) * 512],
                    start=(hh == 0), stop=(hh == NDO - 1))
            osb = osbpool.tile([P, 512], F32, tag="osb", name="osb")
            nc.scalar.activation(osb[:], ps[:], COPYF)
            r0 = sqt * SQT + sc * P
            nc.sync.dma_start(out[r0:r0 + P, do * 512:(do + 1) * 512], osb[:])

        if True:
            # K projection: xk load trains are issued up front so the casting
            # DMAs and XBAR transposes stream back-to-back; weight and xq
            # loads follow in consumption order.
            wk_sb = load_weight(wk, DIN, DC, "wk_sb")
            bk_sb = const.tile([P, NDO], F32)
            nc.sync.dma_start(bk_sb[:], bk[:].rearrange("(o p) -> p o", p=P))
            bq_sb = const.tile([P, NDO], F32)
            nc.sync.dma_start(bq_sb[:], bq[:].rearrange("(o p) -> p o", p=P))
            bv_st = const.tile([1, DC], F32)
            nc.sync.dma_start(bv_st[0:1, :], bv[:][None, :])
            bv_bc = const.tile([P, DC], F32)
            nc.gpsimd.partition_broadcast(bv_bc[:], bv_st[0:1, :])
            NST = S // 512
            xkn = [load_xn(xk, i * 512, 512, "k") for i in range(NST)]
            xkt = [transpose_xn_pe(xkn[i], 512, "k", ps_st)
                   for i in range(NST)]
            for st_i in range(NST):
                xt = xkt[st_i]
                if st_i == 0:
                    wq_sb = load_weight(wq, DIN, DC, "wq_sb")
                    xn_q0 = load_xn(xq, 0, SQT, "q", npool=xqnpool)
                elif st_i == 1:
                    xt_q0 = transpose_xn_pe(xn_q0, SQT, "q", ps_st,
                                            tpool=xqtpool)
                elif st_i == 2:
                    wv_sb = load_weight(wv, DIN, DC, "wv_sb")
                elif st_i == 3:
                    xvn_pre = [load_xn(xv, 0, 512, "v"),
                               load_xn(xv, 512, 512, "v")]
                if dbg and st_i == 0:
                    dump(d_xt, xt[:])
                if dbg and st_i == 1:
                    dump(d_xt2, xt[:])
                for do in range(NDO):
                    ps = ps_gen.tile([P, 512], F32, tag="gen", name="psk")
                    for kt in range(NDIN):
                        nc.tensor.matmul(
                            ps[:], wk_sb[:, kt, do * P:(do + 1) * P],
                            xt[:, kt, :],
                            start=(kt == 0), stop=(kt == NDIN - 1))
                    nc.scalar.activation(
                        KT[:, do, st_i * 512:(st_i + 1) * 512], ps[:],
                        IDENT, bias=bk_sb[:, do:do + 1])

            # V projection is deferred: chunk c is produced as a PE filler
            # inside attention head 0 of sqt 0, just ahead of the OT matmul
            # that consumes it; xv load trains are issued before it starts.
            QT0 = qpool.tile([P, NDO, SQT], BF16, tag="qt", name="qt")
            for do in range(NDO):
                qproj_group(xt_q0, QT0, do)
            xvn = xvn_pre
            xvt_tiles = {}
            for i in range(S // 512):
                if i + 2 < S // 512:
                    xvn.append(load_xn(xv, (i + 2) * 512, 512, "v"))
                xvt_tiles[i] = transpose_xn_pe(xvn[i], 512, "v", ps_st)
            del xvn

            def vchunk_group(c):
                st_i, sc = c // 4, c % 4
                xtv = xvt_tiles[st_i]
                ps = ps_gen.tile([P, 512], F32, tag="gen", name="psv")
                psd = ps[:, :DC]
                for kt in range(NDIN):
                    nc.tensor.matmul(
                        psd[:], xtv[:, kt, sc * P:(sc + 1) * P],
                        wv_sb[:, kt, :],
                        start=(kt == 0), stop=(kt == NDIN - 1))
                nc.vector.tensor_tensor(
                    V[:, c, :, 0:depth],
                    psd[:].rearrange("p (h d) -> p h d", h=H),
                    bv_bc[:].rearrange("p (h d) -> p h d", h=H),
                    mybir.AluOpType.add)

        QT_cur = QT0
        if dbg:
            dump(d_kt, KT[:, :, 0:512])
            dump(d_v, V[:, 0, :, :])
            dump(d_kt2, KT[:, :, 512:1024])
            dump(d_v2, V[:, 5, :, :])
            dump(d_qt, QT_cur[:])

        OTn_prev = None
        pend_norm = None
        for sqt in range(NSQT):
            OTn = otnpool.tile([P, NDO, SQT], BF16, tag="otn", name="otn")
            if sqt + 1 < NSQT:
                xt_q, QT_next = qproj_start(sqt + 1)
            else:
                QT_next = None

            for h in range(H):
                pend_norm = attn_head(
                    QT_cur, OTn, h,
                    dump_first=(dbg and sqt == 0 and h == 0),
                    kt_filler=(vchunk_group if sqt == 0 and h == 0
                               else None),
                    norm_filler=pend_norm)
                if sqt == 0 and h == 2:
                    wo_sb = load_weight(wo, DC, DOUT, "wo_sb")
                # interleave projection fillers between heads
                if h % 2 == 0 and QT_next is not None:
                    qproj_group(xt_q, QT_next, h // 2)
                elif OTn_prev is not None and h % 2 == 1:
                    outproj_group(OTn_prev, sqt - 1, h - 1)
                    outproj_group(OTn_prev, sqt - 1, h)
            if dbg and sqt == 0:
                dump(d_otn, OTn[:])

            OTn_prev, QT_cur = OTn, QT_next

        pend_norm()
        # final sqt's out-projection
        for g in range(2 * NQC):
            outproj_group(OTn_prev, NSQT - 1, g)

    nc.compile()
    return nc


# ---------------------------------------------------------------------------
# Host-side wrapper: shard across 8 NeuronCores, run SPMD, gather.
# Core c handles batch b = c // 2 and head-group g = c % 2 (8 of 16 heads,
# i.e. columns [g*512, (g+1)*512) of Wq/Wk/Wv and rows of Wo).
# ---------------------------------------------------------------------------

import numpy as np

from concourse.bass_utils import run_bass_kernel_spmd

_NC = None


def _get_nc():
    global _NC
    if _NC is None:
        _NC = build_mha_core(S=2048, DIN=1024, DC=512, DOUT=1024, H=8,
                             depth=64, num_devices=8)
    return _NC


def _in_maps(q, k, v, Wq, bq, Wk, bk, Wv, bv, Wo, bo):
    f32 = np.float32
    maps = []
    for c in range(8):
        b, g = c // 2, c % 2
        sl = slice(g * 512, (g + 1) * 512)
        maps.append({
            "xq": np.ascontiguousarray(q[b], dtype=f32),
            "xk": np.ascontiguousarray(k[b], dtype=f32),
            "xv": np.ascontiguousarray(v[b], dtype=f32),
            "wq": np.ascontiguousarray(Wq[:, sl], dtype=f32),
            "wk": np.ascontiguousarray(Wk[:, sl], dtype=f32),
            "wv": np.ascontiguousarray(Wv[:, sl], dtype=f32),
            "wo": np.ascontiguousarray(Wo[sl, :], dtype=f32),
            "bq": np.ascontiguousarray(bq[sl], dtype=f32),
            "bk": np.ascontiguousarray(bk[sl], dtype=f32),
            "bv": np.ascontiguousarray(bv[sl], dtype=f32),
        })
    return maps


def _gather(results, bo):
    out = np.empty((4, 2048, 1024), dtype=np.float32)
    bo32 = np.asarray(bo, dtype=np.float32)
    for b in range(4):
        out[b] = results[2 * b]["out"] + results[2 * b + 1]["out"] + bo32
    return out


def kernel(q, k, v, Wq, bq, Wk, bk, Wv, bv, Wo, bo, _trace=False):
    nc = _get_nc()
    res = run_bass_kernel_spmd(
        nc, _in_maps(q, k, v, Wq, bq, Wk, bk, Wv, bv, Wo, bo),
        core_ids=list(range(8)), trace=_trace)
    out = _gather(res.results, bo)
    if _trace:
        kernel.last_results = res
    return out


# revision 53
# speedup vs baseline: 1.1321x; 1.0076x over previous
"""Bass/Tile multi-head attention kernel for TRN2 (v2, all-bf16 datapath).

Per-core problem (core c handles batch b=c//2, head-group g=c%2):
  inputs:  xq, xk, xv [S, DIN] f32      (batch b slices of q/k/v)
           wq, wk, wv [DIN, DC] f32     (column slice for this head group)
           wo [DC, DOUT] f32            (row slice)
           bq, bk, bv [DC] f32
  output:  out [S, DOUT] f32  partial:  host sums the two head-group partials
           per batch and adds bo.

Key structure (H=8 local heads, depth=64, P=128):
  - x / weights enter SBUF as bf16 via gpsimd casting DMAs; x is transposed
    by the DMA XBAR (dma_start_transpose), so the PE does no transposes.
  - KT/QT are packed 2-heads-per-128-partition blocks; per-head matmuls use
    64-partition views at base 0/64 (contraction over depth=64).
  - ST:  st[k, q] = KT_h^T @ QT_h        (N=512 moving)
  - exp: split between ACT (exact, cols 0:QA) and DVE (Schraudolph int16
    bit-trick -> bf16 bits, cols QA:512), software-pipelined pdepth deep.
  - OT:  ot[d+1, q] += V_aug_h^T @ ex    (V stationary; row 64 of V_aug is
         ones -> softmax denominator in ot[64, :])
  - normalize: reciprocal + partition_broadcast + multiply, written into
    OTn [dc, q] bf16 (plain tensor_copy shifts partition base, HW-proven).
  - V-projection chunks are deferred into attention head 0 of sqt 0 as PE
    fillers; qproj(sqt+1)/outproj(sqt-1) fill between later heads.
"""

from contextlib import ExitStack

import concourse.mybir as mybir
from concourse import bacc
from concourse.tile import TileContext

F32 = mybir.dt.float32
BF16 = mybir.dt.bfloat16
I16 = mybir.dt.int16
P = 128
EXP = mybir.ActivationFunctionType.Exp
IDENT = mybir.ActivationFunctionType.Identity
COPYF = mybir.ActivationFunctionType.Copy
LOG2E = 1.4426950408889634


def build_mha_core(S=2048, DIN=1024, DC=512, DOUT=1024, H=8, depth=64,
                   SQT=512, KG=1, QA=352, num_devices=1,
                   st_bufs=4, ex_bufs=7, ot_bufs=2, gen_bufs=2, pdepth=6,
                   dbg=False):
    assert DC == H * depth and DC % P == 0 and DIN % P == 0 and S % SQT == 0
    NKT = S // P          # key chunks of 128
    NDIN = DIN // P       # input-dim k-tiles
    NDO = DC // P         # d_core blocks (2 heads each)
    NSQT = S // SQT       # attention q tiles
    NQC = SQT // P        # 128-wide q chunks per sqt
    NKG = NKT // KG
    scale = 1.0 / float(depth) ** 0.5
    a_exp = 128.0 * LOG2E * scale
    b_exp = 16250.4

    nc = bacc.Bacc("TRN2", target_bir_lowering=False, debug=False,
                   num_devices=num_devices)
    xq = nc.dram_tensor("xq", [S, DIN], F32, kind="ExternalInput")
    xk = nc.dram_tensor("xk", [S, DIN], F32, kind="ExternalInput")
    xv = nc.dram_tensor("xv", [S, DIN], F32, kind="ExternalInput")
    wq = nc.dram_tensor("wq", [DIN, DC], F32, kind="ExternalInput")
    wk = nc.dram_tensor("wk", [DIN, DC], F32, kind="ExternalInput")
    wv = nc.dram_tensor("wv", [DIN, DC], F32, kind="ExternalInput")
    wo = nc.dram_tensor("wo", [DC, DOUT], F32, kind="ExternalInput")
    bq = nc.dram_tensor("bq", [DC], F32, kind="ExternalInput")
    bk = nc.dram_tensor("bk", [DC], F32, kind="ExternalInput")
    bv = nc.dram_tensor("bv", [DC], F32, kind="ExternalInput")
    out = nc.dram_tensor("out", [S, DOUT], F32, kind="ExternalOutput")
    if dbg:
        d_xt = nc.dram_tensor("d_xt", [P, 8 * 512], F32, kind="ExternalOutput")
        d_kt = nc.dram_tensor("d_kt", [P, 4 * 512], F32, kind="ExternalOutput")
        d_v = nc.dram_tensor("d_v", [P, 8 * 65], F32, kind="ExternalOutput")
        d_qt = nc.dram_tensor("d_qt", [P, 4 * 512], F32, kind="ExternalOutput")
        d_st = nc.dram_tensor("d_st", [P, 2 * 512], F32, kind="ExternalOutput")
        d_ex = nc.dram_tensor("d_ex", [P, 2 * 512], F32, kind="ExternalOutput")
        d_ot = nc.dram_tensor("d_ot", [P, 512], F32, kind="ExternalOutput")
        d_on = nc.dram_tensor("d_on", [P, 4 * 8 * 64], F32, kind="ExternalOutput")
        d_otn = nc.dram_tensor("d_otn", [P, 4 * 512], F32, kind="ExternalOutput")
        d_kt2 = nc.dram_tensor("d_kt2", [P, 4 * 512], F32, kind="ExternalOutput")
        d_v2 = nc.dram_tensor("d_v2", [P, 8 * 65], F32, kind="ExternalOutput")
        d_xt2 = nc.dram_tensor("d_xt2", [P, 8 * 512], F32, kind="ExternalOutput")

    with TileContext(nc) as tc, ExitStack() as ctx:
        const = ctx.enter_context(tc.tile_pool(name="const", bufs=1))
        wpool = ctx.enter_context(tc.tile_pool(name="wpool", bufs=1))
        kvpool = ctx.enter_context(tc.tile_pool(name="kv", bufs=1))
        xnpool = ctx.enter_context(tc.tile_pool(name="xn", bufs=4))
        xtpool = ctx.enter_context(tc.tile_pool(name="xt", bufs=4))
        xqnpool = ctx.enter_context(tc.tile_pool(name="xqn", bufs=2))
        xqtpool = ctx.enter_context(tc.tile_pool(name="xqt", bufs=2))

        # ---- weights: casting DMA f32 -> bf16, split into <=4KB/partition
        def load_weight(dram, kdim, ndim, name):
            w = wpool.tile([P, kdim // P, ndim], BF16, name=name)
            nc.gpsimd.dma_start(
                w[:], dram[:, :].rearrange("(o p) n -> p o n", p=P))
            return w

        # ---- x loading: casting DMA to bf16 natural layout, then XBAR
        # transpose chunks of 128 rows into [din_part, NDIN, s] layout.
        def load_xn(xdram, r0, nrows, tag, npool=None):
            npool = npool or xnpool
            nch = nrows // P
            xn = npool.tile([P, nch, DIN], BF16, tag="xn", name="xn" + tag)
            nc.gpsimd.dma_start(
                xn[:],
                xdram[r0:r0 + nrows, :].rearrange("(c p) d -> p c d", p=P))
            return xn

        def transpose_xn(xn, nrows, tag, tpool=None):
            tpool = tpool or xtpool
            xt = tpool.tile([P, NDIN, nrows], BF16, tag="xt", name="xt" + tag)
            for c in range(nrows // P):
                nc.sync.dma_start_transpose(
                    xt[:, :, c * P:(c + 1) * P], xn[:, c, :])
            return xt

        def load_xt(xdram, r0, nrows, tag, npool=None, tpool=None):
            xn = load_xn(xdram, r0, nrows, tag, npool)
            return transpose_xn(xn, nrows, tag, tpool)

        def transpose_xn_pe(xn, nrows, tag, psum_pool, tpool=None):
            # PE-side transpose: 53ns per 128x128 bf16 tile, frees the DMA
            # engines during the front phase.  PSUM f32 -> bf16 copy on DVE.
            tpool = tpool or xtpool
            nch = nrows // P
            xt = tpool.tile([P, NDIN, nrows], BF16, tag="xt", name="xt" + tag)
            for dblk in range(NDIN):
                tp = psum_pool.tile([P, KG, 512], F32, tag="st", name="tpx")
                tpb = tp[:, 0, 0:nrows // 2].bitcast(BF16)
                for c in range(nch):
                    nc.tensor.transpose(
                        tpb[:, c * P:(c + 1) * P],
                        xn[:, c, dblk * P:(dblk + 1) * P], ident[:])
                nc.vector.tensor_copy(xt[:, dblk, :], tpb[:])
            return xt

        from concourse.masks import make_identity
        ident = const.tile([P, P], BF16)
        make_identity(nc, ident)

        # ---- persistent K^T and V ----
        KT = kvpool.tile([P, NDO, S], BF16)
        V = kvpool.tile([P, NKT, H, depth + 1], BF16)
        nc.vector.memset(V[:, :, :, depth:depth + 1], 1.0)

        if dbg:
            dbgpool = ctx.enter_context(tc.tile_pool(name="dbgp", bufs=2))

        def dump(dram, src):
            n = 1
            for d in src.shape[1:]:
                n *= d
            stg = dbgpool.tile([P] + list(src.shape[1:]), F32, tag="dbgs",
                               name="dbgs")
            nc.vector.tensor_copy(stg[:src.shape[0]], src[:])
            nc.sync.dma_start(
                dram[0:src.shape[0], 0:n],
                stg[:src.shape[0]].rearrange(
                    {2: "p a -> p a", 3: "p a b -> p (a b)",
                     4: "p a b c -> p (a b c)"}[len(src.shape)]))

        # ---- attention-phase pools ----
        qpool = ctx.enter_context(tc.tile_pool(name="qp", bufs=2))
        otnpool = ctx.enter_context(tc.tile_pool(name="otn", bufs=2))
        expool = ctx.enter_context(tc.tile_pool(name="ex", bufs=ex_bufs))
        recpool = ctx.enter_context(tc.tile_pool(name="rec", bufs=2))
        osbpool = ctx.enter_context(tc.tile_pool(name="osb", bufs=2))
        ps_st = ctx.enter_context(
            tc.tile_pool(name="ps_st", bufs=st_bufs, space="PSUM"))
        ps_ot = ctx.enter_context(
            tc.tile_pool(name="ps_ot", bufs=ot_bufs, space="PSUM"))
        ps_gen = ctx.enter_context(
            tc.tile_pool(name="ps_gen", bufs=gen_bufs, space="PSUM"))

        def attn_head(QT, OTn, h, dump_first=False, kt_filler=None,
                      norm_filler=None):
            blk, p0 = h // 2, (h % 2) * 64
            ot = ps_ot.tile([depth + 1, SQT], F32, tag="ot", name="ot")
            pend = []  # software pipeline: (ex, kg) waiting for OT emission

            def emit_ot(ex, kg):
                for j in range(KG):
                    kt = kg * KG + j
                    nc.tensor.matmul(
                        ot[:], V[:, kt, h, :], ex[:, j, :],
                        start=(kt == 0), stop=(kt == NKT - 1))

            for kg in range(NKG):
                st = ps_st.tile([P, KG, 512], F32, tag="st", name="st")
                for j in range(KG):
                    kt = kg * KG + j
                    nc.tensor.matmul(
                        st[:, j], KT[p0:p0 + 64, blk, kt * P:(kt + 1) * P],
                        QT[p0:p0 + 64, blk, :], start=True, stop=True)
                ex = expool.tile([P, KG, 512], BF16, tag="ex", name="ex")
                if QA > 0:
                    nc.scalar.activation(ex[:, :, 0:QA], st[:, :, 0:QA],
                                         EXP, scale=scale)
                if QA < 512:
                    nc.vector.tensor_scalar(
                        ex[:, :, QA:512].bitcast(I16), st[:, :, QA:512],
                        a_exp, b_exp,
                        mybir.AluOpType.mult, mybir.AluOpType.add)
                if dump_first and kg == 3:
                    dump(d_st, st[:])
                    dump(d_ex, ex[:])
                if kt_filler is not None:
                    kt_filler(kg)
                if kg == 1 and norm_filler is not None:
                    # previous head's normalize runs here, behind this head's
                    # first exps, so it never delays the OT pipeline start
                    norm_filler()
                pend.append((ex, kg))
                if len(pend) >= pdepth:
                    emit_ot(*pend.pop(0))
            for pe in pend:
                emit_ot(*pe)
            if dump_first:
                dump(d_ot, ot[:])

            def do_normalize():
                # ot row 64 is the softmax denominator.  Compute at partition
                # base 0; plain tensor_copy shifts bases (HW-proven).
                den = recpool.tile([1, SQT], F32, tag="den", name="den")
                nc.vector.tensor_copy(den[0:1, :], ot[depth:depth + 1, :])
                rec = recpool.tile([1, SQT], F32, tag="rec", name="rec")
                nc.vector.reciprocal(rec[0:1, :], den[0:1, :])
                bc = recpool.tile([64, SQT], F32, tag="bc", name="bc")
                nc.gpsimd.partition_broadcast(bc[0:64, :], rec[0:1, :])
                onorm = recpool.tile([64, SQT], BF16, tag="onorm",
                                     name="onorm")
                nc.vector.tensor_tensor(onorm[0:64, :], ot[0:depth, :],
                                        bc[0:64, :], mybir.AluOpType.mult)
                nc.vector.tensor_copy(OTn[p0:p0 + 64, blk, :], onorm[0:64, :])
            return do_normalize

        # ---- main attention loop over q tiles ----

        def qproj_start(sqt):
            """DMA work for Q tile sqt: load + transpose; returns (xt, QT)."""
            xt = load_xt(xq, sqt * SQT, SQT, "q", npool=xqnpool, tpool=xqtpool)
            QT = qpool.tile([P, NDO, SQT], BF16, tag="qt", name="qt")
            return xt, QT

        def qproj_group(xt, QT, do):
            ps = ps_gen.tile([P, 512], F32, tag="gen", name="psq")
            psq = ps[:, :SQT]
            for kt in range(NDIN):
                nc.tensor.matmul(
                    psq[:], wq_sb[:, kt, do * P:(do + 1) * P], xt[:, kt, :],
                    start=(kt == 0), stop=(kt == NDIN - 1))
            nc.vector.tensor_scalar_add(QT[:, do, :], psq[:],
                                        bq_sb[:, do:do + 1])

        def outproj_group(OTn, sqt, g):
            do, sc = g // NQC, g % NQC
            ps = ps_gen.tile([P, 512], F32, tag="gen", name="pso")
            for hh in range(NDO):
                nc.tensor.matmul(
                    ps[:], OTn[:, hh, sc * P:(sc + 1) * P],
                    wo_sb[:, hh, do * 512:(do + 1) * 512],
                    start=(hh == 0), stop=(hh == NDO - 1))
            osb = osbpool.tile([P, 512], F32, tag="osb", name="osb")
            nc.scalar.activation(osb[:], ps[:], COPYF)
            r0 = sqt * SQT + sc * P
            nc.sync.dma_start(out[r0:r0 + P, do * 512:(do + 1) * 512], osb[:])

        if True:
            # K projection: xk load trains are issued up front so the casting
            # DMAs and XBAR transposes stream back-to-back; weight and xq
            # loads follow in consumption order.
            xkn0 = load_xn(xk, 0, 512, "k")
            wk_sb = load_weight(wk, DIN, DC, "wk_sb")
            bk_sb = const.tile([P, NDO], F32)
            nc.sync.dma_start(bk_sb[:], bk[:].rearrange("(o p) -> p o", p=P))
            bq_sb = const.tile([P, NDO], F32)
            nc.sync.dma_start(bq_sb[:], bq[:].rearrange("(o p) -> p o", p=P))
            bv_st = const.tile([1, DC], F32)
            nc.sync.dma_start(bv_st[0:1, :], bv[:][None, :])
            bv_bc = const.tile([P, DC], F32)
            nc.gpsimd.partition_broadcast(bv_bc[:], bv_st[0:1, :])
            NST = S // 512
            xkn = [xkn0] + [load_xn(xk, i * 512, 512, "k")
                            for i in range(1, NST)]
            xkt = [transpose_xn_pe(xkn[i], 512, "k", ps_st)
                   for i in range(NST)]
            for st_i in range(NST):
                xt = xkt[st_i]
                if st_i == 0:
                    wq_sb = load_weight(wq, DIN, DC, "wq_sb")
                    xn_q0 = load_xn(xq, 0, SQT, "q", npool=xqnpool)
                elif st_i == 1:
                    xt_q0 = transpose_xn_pe(xn_q0, SQT, "q", ps_st,
                                            tpool=xqtpool)
                elif st_i == 2:
                    wv_sb = load_weight(wv, DIN, DC, "wv_sb")
                elif st_i == 3:
                    xvn_pre = [load_xn(xv, 0, 512, "v"),
                               load_xn(xv, 512, 512, "v")]
                if dbg and st_i == 0:
                    dump(d_xt, xt[:])
                if dbg and st_i == 1:
                    dump(d_xt2, xt[:])
                for do in range(NDO):
                    ps = ps_gen.tile([P, 512], F32, tag="gen", name="psk")
                    for kt in range(NDIN):
                        nc.tensor.matmul(
                            ps[:], wk_sb[:, kt, do * P:(do + 1) * P],
                            xt[:, kt, :],
                            start=(kt == 0), stop=(kt == NDIN - 1))
                    nc.scalar.activation(
                        KT[:, do, st_i * 512:(st_i + 1) * 512], ps[:],
                        IDENT, bias=bk_sb[:, do:do + 1])

            # V projection is deferred: chunk c is produced as a PE filler
            # inside attention head 0 of sqt 0, just ahead of the OT matmul
            # that consumes it; xv load trains are issued before it starts.
            QT0 = qpool.tile([P, NDO, SQT], BF16, tag="qt", name="qt")
            for do in range(NDO):
                qproj_group(xt_q0, QT0, do)
            xvn = xvn_pre
            xvt_tiles = {}
            for i in range(S // 512):
                if i + 2 < S // 512:
                    xvn.append(load_xn(xv, (i + 2) * 512, 512, "v"))
                xvt_tiles[i] = transpose_xn_pe(xvn[i], 512, "v", ps_st)
            del xvn

            def vchunk_group(c):
                st_i, sc = c // 4, c % 4
                xtv = xvt_tiles[st_i]
                ps = ps_gen.tile([P, 512], F32, tag="gen", name="psv")
                psd = ps[:, :DC]
                for kt in range(NDIN):
                    nc.tensor.matmul(
                        psd[:], xtv[:, kt, sc * P:(sc + 1) * P],
                        wv_sb[:, kt, :],
                        start=(kt == 0), stop=(kt == NDIN - 1))
                nc.vector.tensor_tensor(
                    V[:, c, :, 0:depth],
                    psd[:].rearrange("p (h d) -> p h d", h=H),
                    bv_bc[:].rearrange("p (h d) -> p h d", h=H),
                    mybir.AluOpType.add)

        QT_cur = QT0
        if dbg:
            dump(d_kt, KT[:, :, 0:512])
            dump(d_v, V[:, 0, :, :])
            dump(d_kt2, KT[:, :, 512:1024])
            dump(d_v2, V[:, 5, :, :])
            dump(d_qt, QT_cur[:])

        OTn_prev = None
        pend_norm = None
        for sqt in range(NSQT):
            OTn = otnpool.tile([P, NDO, SQT], BF16, tag="otn", name="otn")
            if sqt + 1 < NSQT:
                xt_q, QT_next = qproj_start(sqt + 1)
            else:
                QT_next = None

            for h in range(H):
                pend_norm = attn_head(
                    QT_cur, OTn, h,
                    dump_first=(dbg and sqt == 0 and h == 0),
                    kt_filler=(vchunk_group if sqt == 0 and h == 0
                               else None),
                    norm_filler=pend_norm)
                if sqt == 0 and h == 2:
                    wo_sb = load_weight(wo, DC, DOUT, "wo_sb")
                # interleave projection fillers between heads
                if h % 2 == 0 and QT_next is not None:
                    qproj_group(xt_q, QT_next, h // 2)
                elif OTn_prev is not None and h % 2 == 1:
                    outproj_group(OTn_prev, sqt - 1, h - 1)
                    outproj_group(OTn_prev, sqt - 1, h)
            if dbg and sqt == 0:
                dump(d_otn, OTn[:])

            OTn_prev, QT_cur = OTn, QT_next

        pend_norm()
        # final sqt's out-projection
        for g in range(2 * NQC):
            outproj_group(OTn_prev, NSQT - 1, g)

    nc.compile()
    return nc


# ---------------------------------------------------------------------------
# Host-side wrapper: shard across 8 NeuronCores, run SPMD, gather.
# Core c handles batch b = c // 2 and head-group g = c % 2 (8 of 16 heads,
# i.e. columns [g*512, (g+1)*512) of Wq/Wk/Wv and rows of Wo).
# ---------------------------------------------------------------------------

import numpy as np

from concourse.bass_utils import run_bass_kernel_spmd

_NC = None


def _get_nc():
    global _NC
    if _NC is None:
        _NC = build_mha_core(S=2048, DIN=1024, DC=512, DOUT=1024, H=8,
                             depth=64, num_devices=8)
    return _NC


def _in_maps(q, k, v, Wq, bq, Wk, bk, Wv, bv, Wo, bo):
    f32 = np.float32
    maps = []
    for c in range(8):
        b, g = c // 2, c % 2
        sl = slice(g * 512, (g + 1) * 512)
        maps.append({
            "xq": np.ascontiguousarray(q[b], dtype=f32),
            "xk": np.ascontiguousarray(k[b], dtype=f32),
            "xv": np.ascontiguousarray(v[b], dtype=f32),
            "wq": np.ascontiguousarray(Wq[:, sl], dtype=f32),
            "wk": np.ascontiguousarray(Wk[:, sl], dtype=f32),
            "wv": np.ascontiguousarray(Wv[:, sl], dtype=f32),
            "wo": np.ascontiguousarray(Wo[sl, :], dtype=f32),
            "bq": np.ascontiguousarray(bq[sl], dtype=f32),
            "bk": np.ascontiguousarray(bk[sl], dtype=f32),
            "bv": np.ascontiguousarray(bv[sl], dtype=f32),
        })
    return maps


def _gather(results, bo):
    out = np.empty((4, 2048, 1024), dtype=np.float32)
    bo32 = np.asarray(bo, dtype=np.float32)
    for b in range(4):
        out[b] = results[2 * b]["out"] + results[2 * b + 1]["out"] + bo32
    return out


def kernel(q, k, v, Wq, bq, Wk, bk, Wv, bv, Wo, bo, _trace=False):
    nc = _get_nc()
    res = run_bass_kernel_spmd(
        nc, _in_maps(q, k, v, Wq, bq, Wk, bk, Wv, bv, Wo, bo),
        core_ids=list(range(8)), trace=_trace)
    out = _gather(res.results, bo)
    if _trace:
        kernel.last_results = res
    return out
